# revision 5
# baseline (speedup 1.0000x reference)
"""GCN layer (SpMM) Bass kernel for 8 trn2 NeuronCores.

out[i] = sum_{e: rows[e]==i} edge_vals[e] * embeds[cols[e]]
N=100000 nodes, E=1000000 edges, D=64 features.

Strategy: host sorts edges by destination row and splits nodes into 8
contiguous ranges (12500 nodes/core) with disjoint outputs -> no
collectives. Per core, output rows are processed in blocks of 128; each
block's edges are padded to chunks of 128. Per chunk the device does:
  1. indirect DMA gather   emb[p,:]   = embeds[cols[p], :]      (gpsimd)
  2. scale                 embs[p,:]  = emb[p,:] * vals[p]      (scalar)
  3. one-hot               oh[p,r]    = (rrow[p] == r)          (vector)
  4. matmul accumulate     psum[r,:] += oh.T @ embs             (tensor)
After a block's chunks, PSUM is copied to SBUF and DMA'd to the output
rows (contiguous -> plain DMA, no scatter).

The chunk schedule (chunks per block) is computed from the data on the
host and baked into the program; all 8 cores share one program, so the
per-block chunk count is the max over cores (~4% padding).
"""

import sys

import numpy as np

if "/opt/trn_rl_repo" not in sys.path:
    sys.path.insert(0, "/opt/trn_rl_repo")

N_NODES = 100000
D = 64
P = 128
N_CORES = 8


def _build_program(chunks_per_block, n_chunks, n_nodes):
    import concourse.bacc as bacc
    import concourse.bass as bass
    import concourse.tile as tile
    from concourse import mybir

    nodes_per_core = n_nodes // N_CORES
    n_blocks = len(chunks_per_block)

    nc = bacc.Bacc(
        "TRN2",
        target_bir_lowering=False,
        debug=False,
        num_devices=N_CORES,
    )
    embeds_t = nc.dram_tensor("embeds", [n_nodes, D], mybir.dt.float32, kind="ExternalInput")
    cols_t = nc.dram_tensor("cols_p", [P, n_chunks], mybir.dt.int32, kind="ExternalInput")
    vals_t = nc.dram_tensor("vals_p", [P, n_chunks], mybir.dt.float32, kind="ExternalInput")
    rrow_t = nc.dram_tensor("rrow_p", [P, n_chunks], mybir.dt.float32, kind="ExternalInput")
    iota_t = nc.dram_tensor("iota", [P, P], mybir.dt.float32, kind="ExternalInput")
    out_t = nc.dram_tensor("out", [n_blocks * P, D], mybir.dt.float32, kind="ExternalOutput")

    with tile.TileContext(nc) as tc:
        with (
            tc.tile_pool(name="static", bufs=1) as static_pool,
            tc.tile_pool(name="emb", bufs=8) as emb_pool,
            tc.tile_pool(name="sc", bufs=4) as sc_pool,
            tc.tile_pool(name="oh", bufs=4) as oh_pool,
            tc.tile_pool(name="outp", bufs=4) as out_pool,
            tc.tile_pool(name="psum", bufs=4, space="PSUM") as psum_pool,
        ):
            cols_sb = static_pool.tile([P, n_chunks], mybir.dt.int32)
            vals_sb = static_pool.tile([P, n_chunks], mybir.dt.float32)
            rrow_sb = static_pool.tile([P, n_chunks], mybir.dt.float32)
            iota_sb = static_pool.tile([P, P], mybir.dt.float32)
            nc.sync.dma_start(out=cols_sb[:], in_=cols_t[:])
            nc.sync.dma_start(out=vals_sb[:], in_=vals_t[:])
            nc.sync.dma_start(out=rrow_sb[:], in_=rrow_t[:])
            nc.sync.dma_start(out=iota_sb[:], in_=iota_t[:])

            j = 0
            for b in range(n_blocks):
                nb = int(chunks_per_block[b])
                psum_tile = psum_pool.tile([P, D], dtype=mybir.dt.float32, space="PSUM")
                for t in range(nb):
                    emb_tile = emb_pool.tile([P, D], mybir.dt.float32)
                    nc.gpsimd.indirect_dma_start(
                        out=emb_tile[:],
                        out_offset=None,
                        in_=embeds_t[:],
                        in_offset=bass.IndirectOffsetOnAxis(
                            ap=cols_sb[:, j : j + 1], axis=0
                        ),
                    )
                    embs_tile = sc_pool.tile([P, D], mybir.dt.float32)
                    nc.scalar.activation(
                        out=embs_tile[:],
                        in_=emb_tile[:],
                        func=mybir.ActivationFunctionType.Copy,
                        scale=vals_sb[:, j : j + 1],
                    )
                    oh_tile = oh_pool.tile([P, P], mybir.dt.float32)
                    nc.vector.tensor_tensor(
                        out=oh_tile[:],
                        in0=rrow_sb[:, j : j + 1].to_broadcast([P, P]),
                        in1=iota_sb[:],
                        op=mybir.AluOpType.is_equal,
                    )
                    nc.tensor.matmul(
                        out=psum_tile[:],
                        lhsT=oh_tile[:],
                        rhs=embs_tile[:],
                        start=(t == 0),
                        stop=(t == nb - 1),
                    )
                    j += 1
                o_sb = out_pool.tile([P, D], mybir.dt.float32)
                nc.scalar.copy(out=o_sb[:], in_=psum_tile[:])
                nc.sync.dma_start(out=out_t[b * P : (b + 1) * P, :], in_=o_sb[:])
    nc.compile()
    return nc


def _kernel_impl(rows, cols, edge_vals, embeds, n_nodes, trace=False):
    from concourse.bass_utils import run_bass_kernel_spmd

    rows = np.asarray(rows).astype(np.int64)
    cs_all = np.asarray(cols).astype(np.int32)
    vs_all = np.asarray(edge_vals).astype(np.float32)
    embeds = np.ascontiguousarray(np.asarray(embeds), dtype=np.float32)

    nodes_per_core = n_nodes // N_CORES
    assert nodes_per_core * N_CORES == n_nodes
    n_blocks = (nodes_per_core + P - 1) // P

    order = np.argsort(rows, kind="stable")
    rs = rows[order]
    cs = cs_all[order]
    vs = vs_all[order]

    core_of_edge = rs // nodes_per_core
    blk_of_edge = (rs - core_of_edge * nodes_per_core) // P
    cnt = np.bincount(
        core_of_edge * n_blocks + blk_of_edge, minlength=N_CORES * n_blocks
    ).reshape(N_CORES, n_blocks)

    chunks_per_block = np.maximum(1, -(-cnt.max(axis=0) // P))  # ceil div
    n_chunks = int(chunks_per_block.sum())
    chunk_base = np.concatenate([[0], np.cumsum(chunks_per_block)])

    cols_p = np.zeros((N_CORES, n_chunks * P), np.int32)
    vals_p = np.zeros((N_CORES, n_chunks * P), np.float32)
    rrow_p = np.zeros((N_CORES, n_chunks * P), np.float32)
    core_edge_bounds = np.searchsorted(rs, np.arange(0, n_nodes + 1, nodes_per_core))
    for k in range(N_CORES):
        e0 = int(core_edge_bounds[k])
        for b in range(n_blocks):
            c = int(cnt[k, b])
            s = int(chunk_base[b]) * P
            cols_p[k, s : s + c] = cs[e0 : e0 + c]
            vals_p[k, s : s + c] = vs[e0 : e0 + c]
            rrow_p[k, s : s + c] = (
                rs[e0 : e0 + c] - k * nodes_per_core - b * P
            ).astype(np.float32)
            e0 += c

    # device layout: [P, n_chunks], partition p / chunk j <- edge j*P+p
    def dev(a, dt):
        return np.ascontiguousarray(
            a.reshape(N_CORES, n_chunks, P).transpose(0, 2, 1)
        ).astype(dt)

    cols_d = dev(cols_p, np.int32)
    vals_d = dev(vals_p, np.float32)
    rrow_d = dev(rrow_p, np.float32)
    iota = np.ascontiguousarray(
        np.tile(np.arange(P, dtype=np.float32), (P, 1))
    )

    nc = _build_program(chunks_per_block, n_chunks, n_nodes)
    in_maps = [
        {
            "embeds": embeds,
            "cols_p": cols_d[k],
            "vals_p": vals_d[k],
            "rrow_p": rrow_d[k],
            "iota": iota,
        }
        for k in range(N_CORES)
    ]
    global _LAST
    _LAST = (nc, in_maps)
    r = run_bass_kernel_spmd(nc, in_maps, list(range(N_CORES)), trace=trace)
    out = np.concatenate(
        [r.results[k]["out"][:nodes_per_core] for k in range(N_CORES)], axis=0
    ).astype(np.float32)
    if trace:
        return out, r
    return out


_LAST = None


def kernel(rows, cols, edge_vals, embeds):
    return _kernel_impl(rows, cols, edge_vals, embeds, N_NODES)
